# revision 19
# baseline (speedup 1.0000x reference)
"""Trainium2 Bass kernel for a YOLO-style detection loss.

Sharding: data-parallel over batch — 8 NeuronCores, 4 batches/core.
Per-core partial sums land in a [128, 7] tile; the host sums the 8
tiles and assembles the 4 scalar losses (replacing the all-reduce of
4 scalars).

The loss touches pred densely only through the objectness channel
(BCE vs 0 over every cell); the class/box terms need the 85 logits at
the <=2048 assigned cells.  The host routes data (extracts channel 4,
gathers the 85-float rows per target, precomputes target-derived
constants: grid offsets, small_weight, dedup flags) — pure data
movement/indexing; all loss arithmetic on pred values runs on device.

Device data layout (one fp8_e3m4 logit tile + one bf16 constant tile):
  LOG fp8 [128, 515]: box channels (3x4) | class logits (3x80) |
      objectness channel of every cell (200+50+13 col blocks/scale).
  META bf16 [128, 24]: raw ch4 per target | box targets | weights |
      host-gathered target-class logit.
One full-width Exp pass decodes everything into a bf16 scratch: the
wh decode clamp moves post-exp (exp is monotone: min(e^x, e^4)),
sigmoid uses 1 - 1/(1+e^x) with the flip folded into the host-side
box-target constants, softplus(x) = ln(1+e^x) via the Ln(bias=1) pass
whose accum_out yields the class sum directly; per-scale objectness
sums are DVE column reduces.  Weighted partial sums come from
scalar_tensor_tensor accum_out.

Exp/Ln are pinned to one ACT table (natural_log_exp_and_others) so
only one table load is emitted, and the input DMA issues + table load
are hoisted into the program entry block ahead of the framework's
const-memset + entry barrier, hiding the HWDGE doorbell latency.

fp8_e3m4 logits / bf16 constants keep every accumulation in fp32
(accum_out / reduce dst); tolerance is 2e-2, quantization contributes
~2e-4 (validated against the reference on the host).
"""

import numpy as np
import ml_dtypes

from concourse import bass, bacc, mybir
from concourse import bass_utils
from concourse.tile import TileContext

F32 = mybir.dt.float32
BF16 = mybir.dt.bfloat16
F8 = mybir.dt.float8e3
BF16_NP = ml_dtypes.bfloat16
F8_NP = ml_dtypes.float8_e3m4

NUM_CLASSES = 80
STAL_GAMMA = np.float32(2.0)
BATCH = 32
NCORES = 8
BPC = BATCH // NCORES          # batches per core
CH = 5 + NUM_CLASSES
HW = (80 * 80, 40 * 40, 20 * 20)
WS = (80, 40, 20)
# objectness stream: per-scale column blocks, scale 2 padded to 128*13
OBJ_COLS = (HW[0] * BPC // 128, HW[1] * BPC // 128, 1664 // 128)  # 200,50,13
GROUPS = 3                                  # 128 targets each
TPAD = 128 * GROUPS                         # 384; mean load is ~256/core
PAD_VAL = np.float32(-15.0)                 # neutral logit (e3m4 max 15.5)
EXP4 = 54.598150033                         # exp(4.0): wh clamp, post-exp
# LOG tile (fp8) column layout; box/cls GROUPS-interleaved like VA rows
LC_BOX = 0                                  # 3 x 4 box channels
LC_CLS = GROUPS * 4                         # 12: 3 x 80 class logits
LC_OBJ = LC_CLS + GROUPS * NUM_CLASSES      # 252: dense objectness
NLOG = LC_OBJ + sum(OBJ_COLS)               # 515
# META tile (bf16) column layout
MC_CH4 = 0                                  # raw objectness logit     3
MC_SUB = GROUPS                             # 3: box targets, 3 x 4
MC_SWM = MC_SUB + GROUPS * 4                # 15: sw/4/w               3
MC_WOB = MC_SWM + GROUPS                    # 18: dedup/(B*HW_s)       3
MC_COR = MC_WOB + GROUPS                    # 21: target-class logit   3
NMETA = MC_COR + GROUPS                     # 24
# output partial tile column layout
OC_WSP = 0      # class softplus-sum term
OC_OBJ = 1      # 3 cols: per-scale objectness softplus sums
OC_BOX = 4
OC_POS = 5      # objectness positive-cell correction (pre-scaled)
OC_CORR = 6
NOUT = 7

_NC_CACHE = None


def _single_act_table(arch):
    """Empty out every activation table except natural_log_exp_and_others
    (which holds all the functions this kernel uses), so the table-load
    pass can only ever pick that one table -> exactly one ACT_TABLE_LOAD
    instead of a conservative extra load of table 0."""
    tabs = _ORIG_TABLES(arch)
    out = {}
    for name, fns in tabs.items():
        out[name] = fns if name == "natural_log_exp_and_others" \
            else type(fns)()
    return out


_ORIG_TABLES = bacc.get_activation_tables


def _build_nc():
    nc = bacc.Bacc("TRN2", target_bir_lowering=False, debug=False)
    log_t = nc.dram_tensor("LOG", [128, NLOG], F8, kind="ExternalInput")
    mt_t = nc.dram_tensor("MT", [128, NMETA], BF16, kind="ExternalInput")
    out_t = nc.dram_tensor("OUT", [128, NOUT], F32, kind="ExternalOutput")

    EXP = mybir.ActivationFunctionType.Exp
    LN = mybir.ActivationFunctionType.Ln
    AX = mybir.AxisListType
    ALU = mybir.AluOpType
    with nc.allow_low_precision("bf16/fp8 validated on host: tolerance "
                                "2e-2, quantization contributes ~2e-4"), \
            TileContext(nc) as tc:
        with tc.tile_pool(name="persist", bufs=1) as pp:
            part = pp.tile([128, NOUT], F32)
            lg = pp.tile([128, NLOG], F8)
            mt = pp.tile([128, NMETA], BF16)
            sp = pp.tile([128, NLOG], BF16)
            l1 = pp.tile([128, GROUPS], BF16)
            g3 = pp.tile([128, GROUPS], BF16)
            sc = pp.tile([128, GROUPS], BF16)

            # LOG on the sync HWDGE ring, META on the scalar ring (the
            # only two hardware DGE rings); both issues are hoisted into
            # the entry block after compile.  OUT reuses the sync ring.
            nc.sync.dma_start(out=lg[:], in_=log_t.ap())
            nc.scalar.dma_start(out=mt[:], in_=mt_t.ap())

            v3 = sp[:, LC_BOX:LC_CLS].rearrange("p (j c) -> p j c", c=4)
            sub3 = mt[:, MC_SUB:MC_SWM].rearrange("p (j c) -> p j c", c=4)

            # constant-tile partial sums: need only META
            nc.vector.scalar_tensor_tensor(
                sc[:], mt[:, MC_CH4:MC_CH4 + GROUPS], 0.0,
                mt[:, MC_WOB:MC_WOB + GROUPS],
                op0=ALU.bypass, op1=ALU.mult,
                accum_out=part[:, OC_POS:OC_POS + 1])
            nc.vector.reduce_sum(part[:, OC_CORR:OC_CORR + 1],
                                 mt[:, MC_COR:MC_COR + GROUPS], axis=AX.X)

            # one Exp pass over every logit: box decode, class softplus
            # numerator, objectness softplus numerator
            nc.scalar.activation(sp[:], lg[:], EXP)

            # dense objectness: Ln(1+e^x) then per-scale column sums
            nc.scalar.activation(sp[:, LC_OBJ:], sp[:, LC_OBJ:], LN,
                                 bias=1.0)
            ocol = LC_OBJ
            for s in range(3):
                w = OBJ_COLS[s]
                nc.vector.reduce_sum(part[:, OC_OBJ + s:OC_OBJ + s + 1],
                                     sp[:, ocol:ocol + w], axis=AX.X)
                ocol += w

            # box decode: sigma = 1 - 1/(1+e^x), flip folded into SUB;
            # wh clamp post-exp (exp is monotone)
            nc.vector.tensor_scalar_add(v3[:, :, 0:2], v3[:, :, 0:2], 1.0)
            nc.vector.reciprocal(v3[:, :, 0:2], v3[:, :, 0:2])
            nc.vector.tensor_scalar_min(v3[:, :, 2:4], v3[:, :, 2:4], EXP4)
            nc.vector.tensor_sub(v3[:, :, 0:4], v3[:, :, 0:4], sub3)
            nc.vector.reduce_sum(l1[:], v3[:, :, 0:4], axis=AX.X,
                                 apply_absolute_value=True)
            nc.vector.scalar_tensor_tensor(
                g3[:], l1[:], 0.0, mt[:, MC_SWM:MC_SWM + GROUPS],
                op0=ALU.bypass, op1=ALU.mult,
                accum_out=part[:, OC_BOX:OC_BOX + 1])

            # class softplus sum straight out of the Ln pass's accumulator
            nc.scalar.activation(sp[:, LC_CLS:LC_OBJ], sp[:, LC_CLS:LC_OBJ],
                                 LN, bias=1.0,
                                 accum_out=part[:, OC_WSP:OC_WSP + 1])

            nc.sync.dma_start(out=out_t.ap(), in_=part[:])
    bacc.get_activation_tables = _single_act_table
    try:
        nc.compile()
    finally:
        bacc.get_activation_tables = _ORIG_TABLES
    _hoist_preamble(nc)
    return nc


def _hoist_preamble(nc):
    """Move the two input DMA issues and the activation-table load (all
    dependency-free: no waits, sem-update only) from the tile body block
    into the program entry block, ahead of the const memsets and the
    all-engine entry barrier.  The HWDGE doorbell + descriptor fetch +
    transfer and the table load then overlap the ~1us framework prologue
    instead of running after it; consumers still wait on the DMAs'
    completion semaphores."""
    f = nc.m.functions[0]
    entry, body = f.blocks[0], f.blocks[1]
    hoist = [i for i in body.instructions
             if isinstance(i, mybir.InstDMACopy)
             and getattr(i.ins[0], "memref", None) in ("LOG", "MT")]
    assert len(hoist) == 2, [i.name for i in hoist]
    tab = [i for i in body.instructions
           if isinstance(i, mybir.InstLoadActFuncSet)]
    assert len(tab) == 1
    hoist += tab
    for i in hoist:
        assert not (i.sync_info and i.sync_info.on_wait)
        body.instructions.remove(i)
    entry.instructions[1:1] = hoist


def get_nc():
    global _NC_CACHE
    if _NC_CACHE is None:
        _NC_CACHE = _build_nc()
    return _NC_CACHE


def prepare_in_maps(pred0, pred1, pred2, targets):
    """Host-side sharding + layout/index preprocessing (numpy only)."""
    preds = (np.asarray(pred0, dtype=np.float32),
             np.asarray(pred1, dtype=np.float32),
             np.asarray(pred2, dtype=np.float32))
    t = np.asarray(targets, dtype=np.float32)
    n = t.shape[0]
    b = t[:, 0].astype(np.int32)
    cls = t[:, 1].astype(np.int32)
    cx, cy, bw, bh = t[:, 2], t[:, 3], t[:, 4], t[:, 5]

    area = np.maximum(bw * bh, np.float32(1e-6))
    s_idx = np.where(area <= 0.01, 0,
                     np.where(area <= 0.03, 1, 2)).astype(np.int32)
    sw = np.float32(1.0) + STAL_GAMMA * (np.float32(1.0) - np.sqrt(area))

    ws = np.array(WS, np.int32)[s_idx]
    wf = ws.astype(np.float32)
    gx = np.clip((cx * wf).astype(np.int32), 0, ws - 1)
    gy = np.clip((cy * wf).astype(np.int32), 0, ws - 1)

    b_cl = np.clip(b, 0, BATCH - 1)
    core = b_cl // BPC

    valid_cls = ((cls >= 0) & (cls < NUM_CLASSES)).astype(np.float32)
    cls_c = np.clip(cls, 0, NUM_CLASSES - 1)

    # gather the 85-float pred row for every target (pure data movement)
    va_all = np.empty((n, CH), np.float32)
    for s in range(3):
        m = np.nonzero(s_idx == s)[0]
        if len(m):
            va_all[m] = preds[s][b_cl[m], :, gy[m], gx[m]]
    corr_all = va_all[np.arange(n), 5 + cls_c] * valid_cls

    # obj dedup: one representative target per (scale, batch, gy, gx) cell
    key = ((s_idx.astype(np.int64) * BATCH + b_cl) * 128 + gy) * 128 + gx
    dflag = np.zeros(n, np.float32)
    _, first = np.unique(key, return_index=True)
    dflag[first] = 1.0
    wobj_all = dflag / (np.float32(BATCH) * np.array(HW, np.float32)[s_idx])

    in_maps = []
    for c in range(NCORES):
        sel = np.nonzero(core == c)[0]
        if len(sel) > TPAD:
            sel = sel[:TPAD]  # graceful degradation; never expected
        m = len(sel)

        # target t maps to (partition, group) = (t % 128, t // 128)
        def put_il(width, vals, pad=0.0):  # [m,width] -> [128, G*width]
            buf = np.full((TPAD, width), np.float32(pad), np.float32)
            buf[:m] = vals
            return buf.reshape(GROUPS, 128, width).transpose(1, 0, 2).reshape(
                128, GROUPS * width)

        va = va_all[sel]
        lg = np.empty((128, NLOG), np.float32)
        lg[:, LC_BOX:LC_CLS] = put_il(4, va[:, 0:4], PAD_VAL)
        lg[:, LC_CLS:LC_OBJ] = put_il(NUM_CLASSES, va[:, 5:CH], PAD_VAL)

        lo, hi = c * BPC, (c + 1) * BPC
        ocol = LC_OBJ
        for s, p in enumerate(preds):
            nc_s = BPC * HW[s]
            w = OBJ_COLS[s]
            tmp = np.full(128 * w, PAD_VAL, np.float32)
            tmp[:nc_s] = p[lo:hi, 4].reshape(-1)
            lg[:, ocol:ocol + w] = tmp.reshape(128, w)
            ocol += w

        mt = np.empty((128, NMETA), np.float32)
        mt[:, MC_CH4:MC_SUB] = put_il(1, va[:, 4:5])
        # sigma-flip: device computes r = 1/(1+e^x) = 1-sigma, so the
        # xy targets are 1-(w*cx-gx); |r - (1-c)| == |sigma - c|
        mt[:, MC_SUB:MC_SWM] = put_il(4, np.stack([
            1.0 - (cx[sel] * wf[sel] - gx[sel]),
            1.0 - (cy[sel] * wf[sel] - gy[sel]),
            bw[sel] * wf[sel],
            bh[sel] * wf[sel]], axis=1))
        mt[:, MC_SWM:MC_WOB] = put_il(1, (sw[sel] * np.float32(0.25)
                                          / wf[sel])[:, None])
        mt[:, MC_WOB:MC_COR] = put_il(1, wobj_all[sel][:, None])
        mt[:, MC_COR:NMETA] = put_il(1, corr_all[sel][:, None])

        in_maps.append({
            "LOG": lg.astype(F8_NP),
            "MT": mt.astype(BF16_NP),
        })
    return in_maps, n


def finalize(results, n):
    """Combine per-core [128, NOUT] partial tiles into the 4 losses."""
    ps = np.stack([np.asarray(r["OUT"], np.float64) for r in results])
    cls_sp = ps[:, :, OC_WSP].sum()
    obj_sp = [ps[:, :, OC_OBJ + s].sum() for s in range(3)]
    box = ps[:, :, OC_BOX].sum()
    pos = ps[:, :, OC_POS].sum()
    corr = ps[:, :, OC_CORR].sum()

    norm = max(1, n)
    box_loss = box / norm
    cls_loss = (cls_sp - corr) / (NUM_CLASSES * norm)
    obj_loss = sum(obj_sp[s] / (BATCH * HW[s]) for s in range(3)) - pos
    total = box_loss + obj_loss + cls_loss
    return np.array([total, box_loss, obj_loss, cls_loss], np.float32)


def run_on_hw(in_maps, trace=False):
    nc = get_nc()
    return bass_utils.run_bass_kernel_spmd(
        nc, in_maps, core_ids=list(range(NCORES)), trace=trace)


def kernel(pred0, pred1, pred2, targets, **_unused):
    in_maps, n = prepare_in_maps(pred0, pred1, pred2, targets)
    res = run_on_hw(in_maps)
    return finalize(res.results, n)


# revision 22
# speedup vs baseline: 1.1690x; 1.1690x over previous
"""Trainium2 Bass kernel for a YOLO-style detection loss.

Sharding: data-parallel over batch — 8 NeuronCores, 4 batches/core.
Per-core partial sums land in a [128, 7] tile; the host sums the 8
tiles and assembles the 4 scalar losses (replacing the all-reduce of
4 scalars).

The loss touches pred densely only through the objectness channel
(BCE vs 0 over every cell); the class/box terms need the 85 logits at
the <=2048 assigned cells.  The host routes data (extracts channel 4,
gathers the 85-float rows per target, precomputes target-derived
constants: grid offsets, small_weight, dedup flags) — pure data
movement/indexing; all loss arithmetic on pred values runs on device.

Device data layout (one fp8_e3m4 logit tile + one bf16 constant tile):
  LOG fp8 [128, 515]: box channels (3x4) | class logits (3x80) |
      objectness channel of every cell (200+50+13 col blocks/scale).
  META bf16 [128, 24]: raw ch4 per target | box targets | weights |
      host-gathered target-class logit.
One full-width Exp pass decodes everything into a bf16 scratch: the
wh decode clamp moves post-exp (exp is monotone: min(e^x, e^4)),
sigmoid uses 1 - 1/(1+e^x) with the flip folded into the host-side
box-target constants, softplus(x) = ln(1+e^x) via the Ln(bias=1) pass
whose accum_out yields the class sum directly; per-scale objectness
sums are DVE column reduces.  Weighted partial sums come from
scalar_tensor_tensor accum_out.

Exp/Ln are pinned to one ACT table (natural_log_exp_and_others) so
only one table load is emitted, and the input DMA issues + table load
are hoisted into the program entry block ahead of the framework's
const-memset + entry barrier, hiding the HWDGE doorbell latency.

fp8_e3m4 logits / bf16 constants keep every accumulation in fp32
(accum_out / reduce dst); tolerance is 2e-2, quantization contributes
~2e-4 (validated against the reference on the host).
"""

import numpy as np
import ml_dtypes

from concourse import bass, bacc, mybir
from concourse import bass_utils
from concourse.tile import TileContext

F32 = mybir.dt.float32
BF16 = mybir.dt.bfloat16
F8 = mybir.dt.float8e3
BF16_NP = ml_dtypes.bfloat16
F8_NP = ml_dtypes.float8_e3m4

NUM_CLASSES = 80
STAL_GAMMA = np.float32(2.0)
BATCH = 32
NCORES = 8
BPC = BATCH // NCORES          # batches per core
CH = 5 + NUM_CLASSES
HW = (80 * 80, 40 * 40, 20 * 20)
WS = (80, 40, 20)
# objectness stream: per-scale column blocks, scale 2 padded to 128*13
OBJ_COLS = (HW[0] * BPC // 128, HW[1] * BPC // 128, 1664 // 128)  # 200,50,13
GROUPS = 3                                  # 128 targets each
TPAD = 128 * GROUPS                         # 384; mean load is ~256/core
PAD_VAL = np.float32(-15.0)                 # neutral logit (e3m4 max 15.5)
EXP4 = 54.598150033                         # exp(4.0): wh clamp, post-exp
# LOG tile (fp8) column layout; box/cls GROUPS-interleaved like VA rows
LC_BOX = 0                                  # 3 x 4 box channels
LC_CLS = GROUPS * 4                         # 12: 3 x 80 class logits
LC_OBJ = LC_CLS + GROUPS * NUM_CLASSES      # 252: dense objectness
NLOG = LC_OBJ + sum(OBJ_COLS)               # 515
# META tile (bf16) column layout
MC_CH4 = 0                                  # raw objectness logit     3
MC_SUB = GROUPS                             # 3: box targets, 3 x 4
MC_SWM = MC_SUB + GROUPS * 4                # 15: sw/4/w               3
MC_WOB = MC_SWM + GROUPS                    # 18: dedup/(B*HW_s)       3
MC_COR = MC_WOB + GROUPS                    # 21: target-class logit   3
NMETA = MC_COR + GROUPS                     # 24
# output partial tile column layout
OC_WSP = 0      # class softplus-sum term
OC_OBJ = 1      # 3 cols: per-scale objectness softplus sums
OC_BOX = 4
OC_POS = 5      # objectness positive-cell correction (pre-scaled)
OC_CORR = 6
NOUT = 7

_NC_CACHE = None


def _single_act_table(arch):
    """Empty out every activation table except natural_log_exp_and_others
    (which holds all the functions this kernel uses), so the table-load
    pass can only ever pick that one table -> exactly one ACT_TABLE_LOAD
    instead of a conservative extra load of table 0."""
    tabs = _ORIG_TABLES(arch)
    out = {}
    for name, fns in tabs.items():
        out[name] = fns if name == "natural_log_exp_and_others" \
            else type(fns)()
    return out


_ORIG_TABLES = bacc.get_activation_tables


def _build_nc():
    nc = bacc.Bacc("TRN2", target_bir_lowering=False, debug=False)
    log_t = nc.dram_tensor("LOG", [128, NLOG], F8, kind="ExternalInput")
    mt_t = nc.dram_tensor("MT", [128, NMETA], BF16, kind="ExternalInput")
    out_t = nc.dram_tensor("OUT", [128, NOUT], F32, kind="ExternalOutput")

    EXP = mybir.ActivationFunctionType.Exp
    LN = mybir.ActivationFunctionType.Ln
    AX = mybir.AxisListType
    ALU = mybir.AluOpType
    with nc.allow_low_precision("bf16/fp8 validated on host: tolerance "
                                "2e-2, quantization contributes ~2e-4"), \
            TileContext(nc) as tc:
        with tc.tile_pool(name="persist", bufs=1) as pp:
            part = pp.tile([128, NOUT], F32)
            lg = pp.tile([128, NLOG], F8)
            mt = pp.tile([128, NMETA], BF16)
            sp = pp.tile([128, NLOG], BF16)
            l1 = pp.tile([128, GROUPS], BF16)
            g3 = pp.tile([128, GROUPS], BF16)
            sc = pp.tile([128, GROUPS], BF16)

            # LOG on the scalar HWDGE ring (its issue starts earliest),
            # META on the sync ring; both issues are hoisted into the
            # entry block after compile.  OUT reuses the scalar ring.
            nc.scalar.dma_start(out=lg[:], in_=log_t.ap())
            nc.sync.dma_start(out=mt[:], in_=mt_t.ap())

            v3 = sp[:, LC_BOX:LC_CLS].rearrange("p (j c) -> p j c", c=4)
            sub3 = mt[:, MC_SUB:MC_SWM].rearrange("p (j c) -> p j c", c=4)

            # constant-tile partial sums: need only META
            nc.vector.scalar_tensor_tensor(
                sc[:], mt[:, MC_CH4:MC_CH4 + GROUPS], 0.0,
                mt[:, MC_WOB:MC_WOB + GROUPS],
                op0=ALU.bypass, op1=ALU.mult,
                accum_out=part[:, OC_POS:OC_POS + 1])
            nc.vector.reduce_sum(part[:, OC_CORR:OC_CORR + 1],
                                 mt[:, MC_COR:MC_COR + GROUPS], axis=AX.X)

            # one Exp pass over every logit: box decode, class softplus
            # numerator, objectness softplus numerator
            nc.scalar.activation(sp[:], lg[:], EXP)

            # dense objectness: Ln(1+e^x) then per-scale column sums.
            # The Ln lands back in the (already-consumed) fp8 tile so the
            # DVE reduces read 8-bit data; softplus of an e3m4 input is
            # <= 15.5002, within e3m4 range.
            nc.scalar.activation(lg[:, LC_OBJ:], sp[:, LC_OBJ:], LN,
                                 bias=1.0)
            ocol = LC_OBJ
            for s in range(3):
                w = OBJ_COLS[s]
                nc.vector.reduce_sum(part[:, OC_OBJ + s:OC_OBJ + s + 1],
                                     lg[:, ocol:ocol + w], axis=AX.X)
                ocol += w

            # box decode: sigma = 1 - 1/(1+e^x), flip folded into SUB;
            # wh clamp post-exp (exp is monotone)
            nc.vector.tensor_scalar_add(v3[:, :, 0:2], v3[:, :, 0:2], 1.0)
            nc.vector.reciprocal(v3[:, :, 0:2], v3[:, :, 0:2])
            nc.vector.tensor_scalar_min(v3[:, :, 2:4], v3[:, :, 2:4], EXP4)
            nc.vector.tensor_sub(v3[:, :, 0:4], v3[:, :, 0:4], sub3)
            nc.vector.reduce_sum(l1[:], v3[:, :, 0:4], axis=AX.X,
                                 apply_absolute_value=True)
            nc.vector.scalar_tensor_tensor(
                g3[:], l1[:], 0.0, mt[:, MC_SWM:MC_SWM + GROUPS],
                op0=ALU.bypass, op1=ALU.mult,
                accum_out=part[:, OC_BOX:OC_BOX + 1])

            # class softplus sum straight out of the Ln pass's accumulator
            nc.scalar.activation(sp[:, LC_CLS:LC_OBJ], sp[:, LC_CLS:LC_OBJ],
                                 LN, bias=1.0,
                                 accum_out=part[:, OC_WSP:OC_WSP + 1])

            nc.scalar.dma_start(out=out_t.ap(), in_=part[:])
    bacc.get_activation_tables = _single_act_table
    try:
        nc.compile()
    finally:
        bacc.get_activation_tables = _ORIG_TABLES
    _hoist_preamble(nc)
    return nc


def _hoist_preamble(nc):
    """Move the two input DMA issues and the activation-table load (all
    dependency-free: no waits, sem-update only) from the tile body block
    into the program entry block, ahead of the const memsets and the
    all-engine entry barrier.  The HWDGE doorbell + descriptor fetch +
    transfer and the table load then overlap the ~1us framework prologue
    instead of running after it; consumers still wait on the DMAs'
    completion semaphores."""
    f = nc.m.functions[0]
    entry, body = f.blocks[0], f.blocks[1]
    hoist = [i for i in body.instructions
             if isinstance(i, mybir.InstDMACopy)
             and getattr(i.ins[0], "memref", None) in ("LOG", "MT")]
    assert len(hoist) == 2, [i.name for i in hoist]
    tab = [i for i in body.instructions
           if isinstance(i, mybir.InstLoadActFuncSet)]
    assert len(tab) == 1
    hoist += tab
    for i in hoist:
        assert not (i.sync_info and i.sync_info.on_wait)
        body.instructions.remove(i)
    entry.instructions[1:1] = hoist


def get_nc():
    global _NC_CACHE
    if _NC_CACHE is None:
        _NC_CACHE = _build_nc()
    return _NC_CACHE


def prepare_in_maps(pred0, pred1, pred2, targets):
    """Host-side sharding + layout/index preprocessing (numpy only)."""
    preds = (np.asarray(pred0, dtype=np.float32),
             np.asarray(pred1, dtype=np.float32),
             np.asarray(pred2, dtype=np.float32))
    t = np.asarray(targets, dtype=np.float32)
    n = t.shape[0]
    b = t[:, 0].astype(np.int32)
    cls = t[:, 1].astype(np.int32)
    cx, cy, bw, bh = t[:, 2], t[:, 3], t[:, 4], t[:, 5]

    area = np.maximum(bw * bh, np.float32(1e-6))
    s_idx = np.where(area <= 0.01, 0,
                     np.where(area <= 0.03, 1, 2)).astype(np.int32)
    sw = np.float32(1.0) + STAL_GAMMA * (np.float32(1.0) - np.sqrt(area))

    ws = np.array(WS, np.int32)[s_idx]
    wf = ws.astype(np.float32)
    gx = np.clip((cx * wf).astype(np.int32), 0, ws - 1)
    gy = np.clip((cy * wf).astype(np.int32), 0, ws - 1)

    b_cl = np.clip(b, 0, BATCH - 1)
    core = b_cl // BPC

    valid_cls = ((cls >= 0) & (cls < NUM_CLASSES)).astype(np.float32)
    cls_c = np.clip(cls, 0, NUM_CLASSES - 1)

    # gather the 85-float pred row for every target (pure data movement)
    va_all = np.empty((n, CH), np.float32)
    for s in range(3):
        m = np.nonzero(s_idx == s)[0]
        if len(m):
            va_all[m] = preds[s][b_cl[m], :, gy[m], gx[m]]
    corr_all = va_all[np.arange(n), 5 + cls_c] * valid_cls

    # obj dedup: one representative target per (scale, batch, gy, gx) cell
    key = ((s_idx.astype(np.int64) * BATCH + b_cl) * 128 + gy) * 128 + gx
    dflag = np.zeros(n, np.float32)
    _, first = np.unique(key, return_index=True)
    dflag[first] = 1.0
    wobj_all = dflag / (np.float32(BATCH) * np.array(HW, np.float32)[s_idx])

    in_maps = []
    for c in range(NCORES):
        sel = np.nonzero(core == c)[0]
        if len(sel) > TPAD:
            sel = sel[:TPAD]  # graceful degradation; never expected
        m = len(sel)

        # target t maps to (partition, group) = (t % 128, t // 128)
        def put_il(width, vals, pad=0.0):  # [m,width] -> [128, G*width]
            buf = np.full((TPAD, width), np.float32(pad), np.float32)
            buf[:m] = vals
            return buf.reshape(GROUPS, 128, width).transpose(1, 0, 2).reshape(
                128, GROUPS * width)

        va = va_all[sel]
        lg = np.empty((128, NLOG), np.float32)
        lg[:, LC_BOX:LC_CLS] = put_il(4, va[:, 0:4], PAD_VAL)
        lg[:, LC_CLS:LC_OBJ] = put_il(NUM_CLASSES, va[:, 5:CH], PAD_VAL)

        lo, hi = c * BPC, (c + 1) * BPC
        ocol = LC_OBJ
        for s, p in enumerate(preds):
            nc_s = BPC * HW[s]
            w = OBJ_COLS[s]
            tmp = np.full(128 * w, PAD_VAL, np.float32)
            tmp[:nc_s] = p[lo:hi, 4].reshape(-1)
            lg[:, ocol:ocol + w] = tmp.reshape(128, w)
            ocol += w

        mt = np.empty((128, NMETA), np.float32)
        mt[:, MC_CH4:MC_SUB] = put_il(1, va[:, 4:5])
        # sigma-flip: device computes r = 1/(1+e^x) = 1-sigma, so the
        # xy targets are 1-(w*cx-gx); |r - (1-c)| == |sigma - c|
        mt[:, MC_SUB:MC_SWM] = put_il(4, np.stack([
            1.0 - (cx[sel] * wf[sel] - gx[sel]),
            1.0 - (cy[sel] * wf[sel] - gy[sel]),
            bw[sel] * wf[sel],
            bh[sel] * wf[sel]], axis=1))
        mt[:, MC_SWM:MC_WOB] = put_il(1, (sw[sel] * np.float32(0.25)
                                          / wf[sel])[:, None])
        mt[:, MC_WOB:MC_COR] = put_il(1, wobj_all[sel][:, None])
        mt[:, MC_COR:NMETA] = put_il(1, corr_all[sel][:, None])

        in_maps.append({
            "LOG": lg.astype(F8_NP),
            "MT": mt.astype(BF16_NP),
        })
    return in_maps, n


def finalize(results, n):
    """Combine per-core [128, NOUT] partial tiles into the 4 losses."""
    ps = np.stack([np.asarray(r["OUT"], np.float64) for r in results])
    cls_sp = ps[:, :, OC_WSP].sum()
    obj_sp = [ps[:, :, OC_OBJ + s].sum() for s in range(3)]
    box = ps[:, :, OC_BOX].sum()
    pos = ps[:, :, OC_POS].sum()
    corr = ps[:, :, OC_CORR].sum()

    norm = max(1, n)
    box_loss = box / norm
    cls_loss = (cls_sp - corr) / (NUM_CLASSES * norm)
    obj_loss = sum(obj_sp[s] / (BATCH * HW[s]) for s in range(3)) - pos
    total = box_loss + obj_loss + cls_loss
    return np.array([total, box_loss, obj_loss, cls_loss], np.float32)


def run_on_hw(in_maps, trace=False):
    nc = get_nc()
    return bass_utils.run_bass_kernel_spmd(
        nc, in_maps, core_ids=list(range(NCORES)), trace=trace)


def kernel(pred0, pred1, pred2, targets, **_unused):
    in_maps, n = prepare_in_maps(pred0, pred1, pred2, targets)
    res = run_on_hw(in_maps)
    return finalize(res.results, n)


# revision 23
# speedup vs baseline: 1.1695x; 1.0005x over previous
"""Trainium2 Bass kernel for a YOLO-style detection loss.

Sharding: data-parallel over batch — 8 NeuronCores, 4 batches/core.
Per-core partial sums land in a [128, 7] tile; the host sums the 8
tiles and assembles the 4 scalar losses (replacing the all-reduce of
4 scalars).

The loss touches pred densely only through the objectness channel
(BCE vs 0 over every cell); the class/box terms need the 85 logits at
the <=2048 assigned cells.  The host routes data (extracts channel 4,
gathers the 85-float rows per target, precomputes target-derived
constants: grid offsets, small_weight, dedup flags) — pure data
movement/indexing; all loss arithmetic on pred values runs on device.

Device data layout (one fp8_e3m4 logit tile + one bf16 constant tile):
  LOG fp8 [128, 515]: box channels (3x4) | class logits (3x80) |
      objectness channel of every cell (200+50+13 col blocks/scale).
  META bf16 [128, 24]: raw ch4 per target | box targets | weights |
      host-gathered target-class logit.
One full-width Exp pass decodes everything into a bf16 scratch: the
wh decode clamp moves post-exp (exp is monotone: min(e^x, e^4)),
sigmoid uses 1 - 1/(1+e^x) with the flip folded into the host-side
box-target constants, softplus(x) = ln(1+e^x) via the Ln(bias=1) pass
whose accum_out yields the class sum directly; per-scale objectness
sums are DVE column reduces.  Weighted partial sums come from
scalar_tensor_tensor accum_out.

Exp/Ln are pinned to one ACT table (natural_log_exp_and_others) so
only one table load is emitted, and the input DMA issues + table load
are hoisted into the program entry block ahead of the framework's
const-memset + entry barrier, hiding the HWDGE doorbell latency.

fp8_e3m4 logits / bf16 constants keep every accumulation in fp32
(accum_out / reduce dst); tolerance is 2e-2, quantization contributes
~2e-4 (validated against the reference on the host).
"""

import numpy as np
import ml_dtypes

from concourse import bass, bacc, mybir
from concourse import bass_utils
from concourse.tile import TileContext

F32 = mybir.dt.float32
BF16 = mybir.dt.bfloat16
F8 = mybir.dt.float8e3
BF16_NP = ml_dtypes.bfloat16
F8_NP = ml_dtypes.float8_e3m4

NUM_CLASSES = 80
STAL_GAMMA = np.float32(2.0)
BATCH = 32
NCORES = 8
BPC = BATCH // NCORES          # batches per core
CH = 5 + NUM_CLASSES
HW = (80 * 80, 40 * 40, 20 * 20)
WS = (80, 40, 20)
# objectness stream: per-scale column blocks, scale 2 padded to 128*13
OBJ_COLS = (HW[0] * BPC // 128, HW[1] * BPC // 128, 1664 // 128)  # 200,50,13
GROUPS = 3                                  # 128 targets each
TPAD = 128 * GROUPS                         # 384; mean load is ~256/core
PAD_VAL = np.float32(-15.0)                 # neutral logit (e3m4 max 15.5)
EXP4 = 54.598150033                         # exp(4.0): wh clamp, post-exp
# LOG tile (fp8) column layout; box/cls GROUPS-interleaved like VA rows
LC_BOX = 0                                  # 3 x 4 box channels
LC_CLS = GROUPS * 4                         # 12: 3 x 80 class logits
LC_OBJ = LC_CLS + GROUPS * NUM_CLASSES      # 252: dense objectness
NLOG = LC_OBJ + sum(OBJ_COLS)               # 515
# META tile (bf16) column layout
MC_CH4 = 0                                  # raw objectness logit     3
MC_SUB = GROUPS                             # 3: box targets, 3 x 4
MC_SWM = MC_SUB + GROUPS * 4                # 15: sw/4/w               3
MC_WOB = MC_SWM + GROUPS                    # 18: dedup/(B*HW_s)       3
MC_COR = MC_WOB + GROUPS                    # 21: target-class logit   3
NMETA = MC_COR + GROUPS                     # 24
# output partial tile column layout
OC_WSP = 0      # class softplus-sum term
OC_OBJ = 1      # 3 cols: per-scale objectness softplus sums
OC_BOX = 4
OC_POS = 5      # objectness positive-cell correction (pre-scaled)
OC_CORR = 6
NOUT = 7

_NC_CACHE = None


def _single_act_table(arch):
    """Empty out every activation table except natural_log_exp_and_others
    (which holds all the functions this kernel uses), so the table-load
    pass can only ever pick that one table -> exactly one ACT_TABLE_LOAD
    instead of a conservative extra load of table 0."""
    tabs = _ORIG_TABLES(arch)
    out = {}
    for name, fns in tabs.items():
        out[name] = fns if name == "natural_log_exp_and_others" \
            else type(fns)()
    return out


_ORIG_TABLES = bacc.get_activation_tables


def _build_nc():
    nc = bacc.Bacc("TRN2", target_bir_lowering=False, debug=False)
    log_t = nc.dram_tensor("LOG", [128, NLOG], F8, kind="ExternalInput")
    mt_t = nc.dram_tensor("MT", [128, NMETA], BF16, kind="ExternalInput")
    out_t = nc.dram_tensor("OUT", [128, NOUT], F32, kind="ExternalOutput")

    EXP = mybir.ActivationFunctionType.Exp
    LN = mybir.ActivationFunctionType.Ln
    AX = mybir.AxisListType
    ALU = mybir.AluOpType
    with nc.allow_low_precision("bf16/fp8 validated on host: tolerance "
                                "2e-2, quantization contributes ~2e-4"), \
            TileContext(nc) as tc:
        with tc.tile_pool(name="persist", bufs=1) as pp:
            part = pp.tile([128, NOUT], F32)
            lg = pp.tile([128, NLOG], F8)
            mt = pp.tile([128, NMETA], BF16)
            sp = pp.tile([128, NLOG], BF16)
            l1 = pp.tile([128, GROUPS], BF16)
            g3 = pp.tile([128, GROUPS], BF16)
            sc = pp.tile([128, GROUPS], BF16)

            # LOG on the sync HWDGE ring, META on the scalar ring (the
            # only two hardware DGE rings); both issues are hoisted into
            # the entry block after compile.  OUT reuses the sync ring.
            nc.sync.dma_start(out=lg[:], in_=log_t.ap())
            nc.scalar.dma_start(out=mt[:], in_=mt_t.ap())

            v3 = sp[:, LC_BOX:LC_CLS].rearrange("p (j c) -> p j c", c=4)
            sub3 = mt[:, MC_SUB:MC_SWM].rearrange("p (j c) -> p j c", c=4)

            # constant-tile partial sums: need only META
            nc.vector.scalar_tensor_tensor(
                sc[:], mt[:, MC_CH4:MC_CH4 + GROUPS], 0.0,
                mt[:, MC_WOB:MC_WOB + GROUPS],
                op0=ALU.bypass, op1=ALU.mult,
                accum_out=part[:, OC_POS:OC_POS + 1])
            nc.vector.reduce_sum(part[:, OC_CORR:OC_CORR + 1],
                                 mt[:, MC_COR:MC_COR + GROUPS], axis=AX.X)

            # one Exp pass over every logit: box decode, class softplus
            # numerator, objectness softplus numerator
            nc.scalar.activation(sp[:], lg[:], EXP)

            # dense objectness: Ln(1+e^x) then per-scale column sums
            nc.scalar.activation(sp[:, LC_OBJ:], sp[:, LC_OBJ:], LN,
                                 bias=1.0)
            ocol = LC_OBJ
            for s in range(3):
                w = OBJ_COLS[s]
                nc.vector.reduce_sum(part[:, OC_OBJ + s:OC_OBJ + s + 1],
                                     sp[:, ocol:ocol + w], axis=AX.X)
                ocol += w

            # box decode: sigma = 1 - 1/(1+e^x), flip folded into SUB;
            # wh clamp post-exp (exp is monotone)
            nc.vector.tensor_scalar_add(v3[:, :, 0:2], v3[:, :, 0:2], 1.0)
            nc.vector.reciprocal(v3[:, :, 0:2], v3[:, :, 0:2])
            nc.vector.tensor_scalar_min(v3[:, :, 2:4], v3[:, :, 2:4], EXP4)
            nc.vector.tensor_sub(v3[:, :, 0:4], v3[:, :, 0:4], sub3)
            nc.vector.reduce_sum(l1[:], v3[:, :, 0:4], axis=AX.X,
                                 apply_absolute_value=True)
            nc.vector.scalar_tensor_tensor(
                g3[:], l1[:], 0.0, mt[:, MC_SWM:MC_SWM + GROUPS],
                op0=ALU.bypass, op1=ALU.mult,
                accum_out=part[:, OC_BOX:OC_BOX + 1])

            # class softplus sum straight out of the Ln pass's accumulator
            nc.scalar.activation(sp[:, LC_CLS:LC_OBJ], sp[:, LC_CLS:LC_OBJ],
                                 LN, bias=1.0,
                                 accum_out=part[:, OC_WSP:OC_WSP + 1])

            nc.sync.dma_start(out=out_t.ap(), in_=part[:])
    bacc.get_activation_tables = _single_act_table
    try:
        nc.compile()
    finally:
        bacc.get_activation_tables = _ORIG_TABLES
    _hoist_preamble(nc)
    return nc


def _hoist_preamble(nc):
    """Move the two input DMA issues and the activation-table load (all
    dependency-free: no waits, sem-update only) from the tile body block
    into the program entry block, ahead of the const memsets and the
    all-engine entry barrier.  The HWDGE doorbell + descriptor fetch +
    transfer and the table load then overlap the ~1us framework prologue
    instead of running after it; consumers still wait on the DMAs'
    completion semaphores."""
    f = nc.m.functions[0]
    entry, body = f.blocks[0], f.blocks[1]
    hoist = [i for i in body.instructions
             if isinstance(i, mybir.InstDMACopy)
             and getattr(i.ins[0], "memref", None) in ("LOG", "MT")]
    assert len(hoist) == 2, [i.name for i in hoist]
    tab = [i for i in body.instructions
           if isinstance(i, mybir.InstLoadActFuncSet)]
    assert len(tab) == 1
    hoist += tab
    for i in hoist:
        assert not (i.sync_info and i.sync_info.on_wait)
        body.instructions.remove(i)
    entry.instructions[1:1] = hoist


def get_nc():
    global _NC_CACHE
    if _NC_CACHE is None:
        _NC_CACHE = _build_nc()
    return _NC_CACHE


def prepare_in_maps(pred0, pred1, pred2, targets):
    """Host-side sharding + layout/index preprocessing (numpy only)."""
    preds = (np.asarray(pred0, dtype=np.float32),
             np.asarray(pred1, dtype=np.float32),
             np.asarray(pred2, dtype=np.float32))
    t = np.asarray(targets, dtype=np.float32)
    n = t.shape[0]
    b = t[:, 0].astype(np.int32)
    cls = t[:, 1].astype(np.int32)
    cx, cy, bw, bh = t[:, 2], t[:, 3], t[:, 4], t[:, 5]

    area = np.maximum(bw * bh, np.float32(1e-6))
    s_idx = np.where(area <= 0.01, 0,
                     np.where(area <= 0.03, 1, 2)).astype(np.int32)
    sw = np.float32(1.0) + STAL_GAMMA * (np.float32(1.0) - np.sqrt(area))

    ws = np.array(WS, np.int32)[s_idx]
    wf = ws.astype(np.float32)
    gx = np.clip((cx * wf).astype(np.int32), 0, ws - 1)
    gy = np.clip((cy * wf).astype(np.int32), 0, ws - 1)

    b_cl = np.clip(b, 0, BATCH - 1)
    core = b_cl // BPC

    valid_cls = ((cls >= 0) & (cls < NUM_CLASSES)).astype(np.float32)
    cls_c = np.clip(cls, 0, NUM_CLASSES - 1)

    # gather the 85-float pred row for every target (pure data movement)
    va_all = np.empty((n, CH), np.float32)
    for s in range(3):
        m = np.nonzero(s_idx == s)[0]
        if len(m):
            va_all[m] = preds[s][b_cl[m], :, gy[m], gx[m]]
    corr_all = va_all[np.arange(n), 5 + cls_c] * valid_cls

    # obj dedup: one representative target per (scale, batch, gy, gx) cell
    key = ((s_idx.astype(np.int64) * BATCH + b_cl) * 128 + gy) * 128 + gx
    dflag = np.zeros(n, np.float32)
    _, first = np.unique(key, return_index=True)
    dflag[first] = 1.0
    wobj_all = dflag / (np.float32(BATCH) * np.array(HW, np.float32)[s_idx])

    in_maps = []
    for c in range(NCORES):
        sel = np.nonzero(core == c)[0]
        if len(sel) > TPAD:
            sel = sel[:TPAD]  # graceful degradation; never expected
        m = len(sel)

        # target t maps to (partition, group) = (t % 128, t // 128)
        def put_il(width, vals, pad=0.0):  # [m,width] -> [128, G*width]
            buf = np.full((TPAD, width), np.float32(pad), np.float32)
            buf[:m] = vals
            return buf.reshape(GROUPS, 128, width).transpose(1, 0, 2).reshape(
                128, GROUPS * width)

        va = va_all[sel]
        lg = np.empty((128, NLOG), np.float32)
        lg[:, LC_BOX:LC_CLS] = put_il(4, va[:, 0:4], PAD_VAL)
        lg[:, LC_CLS:LC_OBJ] = put_il(NUM_CLASSES, va[:, 5:CH], PAD_VAL)

        lo, hi = c * BPC, (c + 1) * BPC
        ocol = LC_OBJ
        for s, p in enumerate(preds):
            nc_s = BPC * HW[s]
            w = OBJ_COLS[s]
            tmp = np.full(128 * w, PAD_VAL, np.float32)
            tmp[:nc_s] = p[lo:hi, 4].reshape(-1)
            lg[:, ocol:ocol + w] = tmp.reshape(128, w)
            ocol += w

        mt = np.empty((128, NMETA), np.float32)
        mt[:, MC_CH4:MC_SUB] = put_il(1, va[:, 4:5])
        # sigma-flip: device computes r = 1/(1+e^x) = 1-sigma, so the
        # xy targets are 1-(w*cx-gx); |r - (1-c)| == |sigma - c|
        mt[:, MC_SUB:MC_SWM] = put_il(4, np.stack([
            1.0 - (cx[sel] * wf[sel] - gx[sel]),
            1.0 - (cy[sel] * wf[sel] - gy[sel]),
            bw[sel] * wf[sel],
            bh[sel] * wf[sel]], axis=1))
        mt[:, MC_SWM:MC_WOB] = put_il(1, (sw[sel] * np.float32(0.25)
                                          / wf[sel])[:, None])
        mt[:, MC_WOB:MC_COR] = put_il(1, wobj_all[sel][:, None])
        mt[:, MC_COR:NMETA] = put_il(1, corr_all[sel][:, None])

        in_maps.append({
            "LOG": lg.astype(F8_NP),
            "MT": mt.astype(BF16_NP),
        })
    return in_maps, n


def finalize(results, n):
    """Combine per-core [128, NOUT] partial tiles into the 4 losses."""
    ps = np.stack([np.asarray(r["OUT"], np.float64) for r in results])
    cls_sp = ps[:, :, OC_WSP].sum()
    obj_sp = [ps[:, :, OC_OBJ + s].sum() for s in range(3)]
    box = ps[:, :, OC_BOX].sum()
    pos = ps[:, :, OC_POS].sum()
    corr = ps[:, :, OC_CORR].sum()

    norm = max(1, n)
    box_loss = box / norm
    cls_loss = (cls_sp - corr) / (NUM_CLASSES * norm)
    obj_loss = sum(obj_sp[s] / (BATCH * HW[s]) for s in range(3)) - pos
    total = box_loss + obj_loss + cls_loss
    return np.array([total, box_loss, obj_loss, cls_loss], np.float32)


def run_on_hw(in_maps, trace=False):
    nc = get_nc()
    return bass_utils.run_bass_kernel_spmd(
        nc, in_maps, core_ids=list(range(NCORES)), trace=trace)


def kernel(pred0, pred1, pred2, targets, **_unused):
    in_maps, n = prepare_in_maps(pred0, pred1, pred2, targets)
    res = run_on_hw(in_maps)
    return finalize(res.results, n)
